# revision 11
# baseline (speedup 1.0000x reference)
"""DilateAttention Trainium2 Bass kernel — v2 (8-unit x 16-channel packing).

Problem: per-pixel 3x3 dilated (dilation=2) local attention.
  q,k,v: [4, 192, 112, 112] f32 ; out: [4, 112, 112, 192] f32
  heads=6, head_dim=32, taps=9, zero-padded windows.

v2 strategy (vs v1's (4 unit x 32 chan) partition groups):
  * 768 flat channels -> 6 groups of 128 = (unit-octet o in {0,1,2}) x
    (channel-half h in {0,1}); within a group partition p = u*16 + c,
    channel = (8o+u)*32 + 16h + c.
  * A unit's 32 channels split across the h=0/h=1 groups; the QK logit
    reduction PSUM-accumulates two matmuls (one per half).  Both halves
    of a (o, row-block) pair run on the same core.
  * Row dim in 8 blocks of 14 rows -> 48 chunks -> 24 (o,e) pairs -> 3
    pairs per core.
  * Wins vs v1: exp processes 8 units per instruction (ACT work halves)
    and the softmax denominator (and its PE tap-sum) is computed once
    per unit instead of once per half; exp taps are pre-paired on
    DVE/GpSimd so the PE D-sum needs 5 passes instead of 9.
  * Product TTs are split across DVE (fp16 2x mode) and GpSimd (idle
    otherwise) to balance engine busy times; PE is the critical path.
  * Inputs are pre-cast to fp16 host-side: input DMAs are non-casting
    (so they queue on SP, not GpSimd) and input HBM traffic halves.
  * Normalization: 1/D via exp(-ln(D)) on ACT, final multiply on DVE
    (DVE has no ISA divide; only one PSUM operand per instruction).
  * k/v zero-padded host-side, reproducing reference softmax boundary
    semantics exactly.
"""

import numpy as np
from contextlib import ExitStack

import concourse.bass as bass
import concourse.tile as tile
from concourse import mybir
from concourse.bass_utils import run_bass_kernel_spmd

# ---------------------------------------------------------------- constants
B, C, H, W = 4, 192, 112, 112
NUM_HEADS, HEAD_DIM = 6, 32
KK = 9
PAD = 2
HP, WP = H + 2 * PAD, W + 2 * PAD  # 116, 116
G = B * C  # 768 flattened channels
N_CORES = 8
CHUNK_ROWS = 14  # rows per chunk (112 / 8)
N_PAIRS_PER_CORE = 3  # 24 (octet, row-block) pairs / 8 cores
SCALE = HEAD_DIM ** -0.5

F16 = mybir.dt.float16
F32 = mybir.dt.float32


def _view(ap, extra_offset, dims):
    """Free-dim access-pattern view on an SBUF/PSUM tile AP."""
    base = ap[:] if not isinstance(ap, bass.AP) else ap
    part = base.ap[0]
    return bass.AP(
        tensor=base.tensor,
        offset=base.offset + extra_offset,
        ap=[part] + [list(d) for d in dims],
    )


def build_nc(reps=1, chunk_rows=CHUNK_ROWS, n_pairs=N_PAIRS_PER_CORE,
             skip=frozenset()):
    """Build the per-core Bass program (SPMD: same program, per-core data)."""
    kr = chunk_rows + 4  # padded k/v rows per chunk
    n_sub = chunk_rows // 2  # 2-row sub-blocks per pair-chunk

    nc = bass.Bass("TRN2", target_bir_lowering=False, debug=False,
                   num_devices=N_CORES)

    q_d = nc.dram_tensor("q", [n_pairs, 128, 2, chunk_rows, W], F16,
                         kind="ExternalInput").ap()
    k_d = nc.dram_tensor("k", [n_pairs, 128, 2, kr, WP], F16,
                         kind="ExternalInput").ap()
    v_d = nc.dram_tensor("v", [n_pairs, 128, 2, kr, WP], F16,
                         kind="ExternalInput").ap()
    bs_d = nc.dram_tensor("bs", [128, 128], F16, kind="ExternalInput").ap()
    id_d = nc.dram_tensor("id128", [128, 128], F16, kind="ExternalInput").ap()
    o_d = nc.dram_tensor("o", [n_pairs, 2, 128, chunk_rows, W], F32,
                         kind="ExternalOutput").ap()

    with tile.TileContext(nc) as tc:
        with ExitStack() as ctx:
            consts = ctx.enter_context(tc.tile_pool(name="consts", bufs=1))
            qpool = ctx.enter_context(
                tc.tile_pool(name="qpool", bufs=2 * n_pairs))
            kpool = ctx.enter_context(
                tc.tile_pool(name="kpool", bufs=2 * n_pairs - 1))
            vpool = ctx.enter_context(
                tc.tile_pool(name="vpool", bufs=2 * n_pairs - 1))
            opool = ctx.enter_context(tc.tile_pool(name="opool", bufs=3))
            prodp = ctx.enter_context(tc.tile_pool(name="prodp", bufs=6))
            cap = ctx.enter_context(tc.tile_pool(name="cap", bufs=2))
            cbp = ctx.enter_context(tc.tile_pool(name="cbp", bufs=2))
            dpool = ctx.enter_context(tc.tile_pool(name="dpool", bufs=4))
            epool = ctx.enter_context(tc.tile_pool(name="epool", bufs=2))
            att_ps = ctx.enter_context(
                tc.tile_pool(name="att_ps", bufs=1, space="PSUM"))
            oda_ps = ctx.enter_context(
                tc.tile_pool(name="oda_ps", bufs=2, space="PSUM"))
            odb_ps = ctx.enter_context(
                tc.tile_pool(name="odb_ps", bufs=1, space="PSUM"))

            bs_sb = consts.tile([128, 128], F16)
            nc.sync.dma_start(out=bs_sb[:], in_=bs_d[:])
            id_sb = consts.tile([128, 128], F16)
            nc.sync.dma_start(out=id_sb[:], in_=id_d[:])

            def body():
                # prefetch all pair chunks (cast f32 -> f16 during DMA);
                # first pair's tensors are loaded in two slices so compute
                # can start while the rest streams in.
                qs, ks, vs = [], [], []
                for t in range(n_pairs):
                    # one tile per tensor holds both groups of the pair
                    # (fewer, bigger DMAs -> less gpsimd queueing time)
                    q_sb = qpool.tile([128, 2, chunk_rows, W], F16, tag="q")
                    k_sb = kpool.tile([128, 2, kr, WP], F16, tag="k")
                    v_sb = vpool.tile([128, 2, kr, WP], F16, tag="v")
                    # inputs are pre-cast to fp16 host-side, so the DMAs
                    # are non-casting and can queue on SP instead of
                    # GpSimd (which is busy with product TTs)
                    for g in range(2):
                        nc.sync.dma_start(out=q_sb[:, g], in_=q_d[t, :, g])
                        nc.sync.dma_start(out=k_sb[:, g], in_=k_d[t, :, g])
                        nc.sync.dma_start(out=v_sb[:, g], in_=v_d[t, :, g])
                    qs.append(q_sb), ks.append(k_sb), vs.append(v_sb)

                for t in range(n_pairs):
                    qg = [_view(qs[t], g * chunk_rows * W,
                                [[W, chunk_rows], [1, W]]) for g in range(2)]
                    kg = [_view(ks[t], g * kr * WP,
                                [[WP, kr], [1, WP]]) for g in range(2)]
                    vg = [_view(vs[t], g * kr * WP,
                                [[WP, kr], [1, WP]]) for g in range(2)]
                    o_sb_a = opool.tile([128, chunk_rows, W], F32, tag="o")
                    o_sb_b = opool.tile([128, chunk_rows, W], F32, tag="o")
                    o_sb = [o_sb_a, o_sb_b]

                    def emit_divides(od_a, od_b, dr, rb):
                        # o = unnorm * (1/D); 1/D comes from ACT via
                        # exp(-ln(D)) (DVE has no ISA divide, and only one
                        # PSUM operand is allowed per instruction)
                        for g, od in ((0, od_a), (1, od_b)):
                            out_v = _view(o_sb[g], rb * W, [[1, 224]])
                            nc.vector.tensor_tensor(
                                out_v, od[:, 0:224], dr[:],
                                mybir.AluOpType.mult)

                    def emit_qk(sb):
                        """QK products of sub-block sb (3 DVE + 1 GpSimd).
                        Called one iteration ahead so the next sub-block's
                        products don't queue behind this one's AV work."""
                        rb = 2 * sb
                        # prod per group: [128, 2(row), 9(kk), 112] fp16
                        prod_a = prodp.tile([128, 2, KK, W], F16, tag="prod")
                        prod_b = prodp.tile([128, 2, KK, W], F16, tag="prod")
                        prods = [prod_a, prod_b]
                        if "qk_tt" not in skip:
                            for g in range(2):
                                for rl in range(2):
                                    if (g, rl) == (1, 1) and \
                                            "pool_av" not in skip:
                                        parts = [(nc.gpsimd, 0, 3)]
                                    elif (g, rl) == (0, 1) and \
                                            "pool_av" not in skip:
                                        # dy 0-1 on DVE, dy 2 on GpSimd
                                        parts = [(nc.vector, 0, 2),
                                                 (nc.gpsimd, 2, 1)]
                                    else:
                                        parts = [(nc.vector, 0, 3)]
                                    for eng, dy0, ndy in parts:
                                        kv = _view(
                                            kg[g],
                                            (rb + rl) * WP + dy0 * 2 * WP,
                                            [[2 * WP, ndy], [2, 3], [1, W]])
                                        qv = _view(
                                            qg[g], (rb + rl) * W,
                                            [[0, ndy], [0, 3], [1, W]])
                                        pv = _view(
                                            prods[g],
                                            rl * KK * W + dy0 * 3 * W,
                                            [[3 * W, ndy], [W, 3], [1, W]])
                                        eng.tensor_tensor(
                                            pv, kv, qv, mybir.AluOpType.mult)
                        return prods

                    pending_div = None
                    prods_cur = emit_qk(0)
                    for sb in range(n_sub):
                        rb = 2 * sb  # first out-row of sub-block
                        prods = prods_cur
                        # c_A: [128, 9, 448] : [tap, 0:224] = A-group AV
                        # products, [tap, 224:448] = exp (shared with B)
                        c_a = cap.tile([128, KK, 448], F16, tag="cA")
                        # c_B: [128, 9, 224] : B-group AV products
                        c_b = cbp.tile([128, KK, 224], F16, tag="cB")
                        # four attn PSUM tiles (5 banks total) so QK mms of
                        # the next sub-block can reuse tile k as soon as
                        # exp k of this one has drained it
                        attn_0 = att_ps.tile([128, 3, 256], F32, tag="att0")
                        attn_1 = att_ps.tile([128, 2, 256], F32, tag="att1")
                        attn_2 = att_ps.tile([128, 2, 256], F32, tag="att2")
                        attn_3 = att_ps.tile([128, 2, 256], F32, tag="att3")
                        att_chunks = ((attn_0, 0, 3), (attn_1, 3, 2),
                                      (attn_2, 5, 2), (attn_3, 7, 2))
                        od_a = oda_ps.tile([128, 448], F32, tag="odA")
                        od_b = odb_ps.tile([128, 224], F32, tag="odB")

                        # ---- logits: per tap, accumulate A then B halves
                        for att_t, k0, nk in att_chunks:
                            if "qk_mm" not in skip:
                                for kl in range(nk):
                                    kk = k0 + kl
                                    for g in range(2):
                                        rhs = _view(prods[g], kk * W,
                                                    [[KK * W, 2], [1, W]])
                                        nc.tensor.matmul(
                                            att_t[:, kl, 0:224], bs_sb[:],
                                            rhs, start=(g == 0),
                                            stop=(g == 1))
                            if "exp" not in skip:
                                attn_v = _view(att_t, 0,
                                               [[256, nk], [1, 224]])
                                exp_out = _view(c_a, k0 * 448 + 224,
                                                [[448, nk], [1, 224]])
                                nc.scalar.activation(
                                    exp_out, attn_v,
                                    mybir.ActivationFunctionType.Exp)

                        # ---- next sub-block's QK products, ahead of this
                        # one's AV work in the DVE/GpSimd queues
                        if sb + 1 < n_sub:
                            prods_cur = emit_qk(sb + 1)

                        # ---- previous sub-block's normalize, slotted into
                        # DVE's idle window between QK and AV products so
                        # it does not head-block the next sub-block
                        if pending_div is not None and "norm" not in skip:
                            emit_divides(*pending_div)
                            pending_div = None

                        # ---- AV products (per group, per row; fused 9
                        # taps).  Row-1 products run on GpSimd (operands
                        # all SBUF, so legal there) to shed DVE load onto
                        # the otherwise idle engine.
                        if "av_tt" not in skip:
                            for g, c_t, cs in ((0, c_a, 448), (1, c_b, 224)):
                                for rl in range(2):
                                    base_v = (rb + rl) * WP
                                    if rl == 1 and "pool_av" not in skip:
                                        # row-1 AV products on GpSimd
                                        parts = [(nc.gpsimd, 0, 3)]
                                    else:
                                        parts = [(nc.vector, 0, 3)]
                                    for eng, dy0, ndy in parts:
                                        vv = _view(
                                            vg[g],
                                            base_v + dy0 * 2 * WP,
                                            [[2 * WP, ndy], [2, 3], [1, W]])
                                        ev = _view(
                                            c_a,
                                            224 + rl * W + dy0 * 3 * 448,
                                            [[3 * 448, ndy], [448, 3],
                                             [1, W]])
                                        p2 = _view(
                                            c_t,
                                            rl * W + dy0 * 3 * cs,
                                            [[3 * cs, ndy], [cs, 3],
                                             [1, W]])
                                        eng.tensor_tensor(
                                            p2, vv, ev, mybir.AluOpType.mult)

                        # ---- pre-pair exp taps (t)+(t+1) for t=0,2,4,6 so
                        # the PE D-sum needs 5 passes instead of 9; one
                        # pair-add on DVE, one on GpSimd
                        epair = epool.tile([128, 4, 224], F16, tag="epair")
                        if "od_mm" not in skip:
                            for eng, p0 in ((nc.vector, 0), (nc.gpsimd, 2)):
                                e0 = _view(c_a, 224 + (2 * p0) * 448,
                                           [[896, 2], [1, 224]])
                                e1 = _view(c_a, 224 + (2 * p0 + 1) * 448,
                                           [[896, 2], [1, 224]])
                                ep = _view(epair, p0 * 224,
                                           [[224, 2], [1, 224]])
                                eng.tensor_tensor(
                                    ep, e0, e1, mybir.AluOpType.add)

                        # ---- tap sums on PE, one accumulation group per
                        # od tile: A gets 9 prod passes into [0:224] plus
                        # 5 D passes into [224:448] (4 pre-paired + tap 8);
                        # B gets prod only (D shared from A)
                        if "od_mm" not in skip:
                            for kk in range(KK):
                                rhs = _view(c_a, kk * 448, [[1, 224]])
                                nc.tensor.matmul(
                                    od_a[:, 0:224], id_sb[:], rhs,
                                    start=(kk == 0), stop=False)
                            for kk in range(4):
                                rhs = _view(epair, kk * 224, [[1, 224]])
                                nc.tensor.matmul(
                                    od_a[:, 224:448], id_sb[:], rhs,
                                    start=False, stop=False)
                            rhs = _view(c_a, 8 * 448 + 224, [[1, 224]])
                            nc.tensor.matmul(
                                od_a[:, 224:448], id_sb[:], rhs,
                                start=False, stop=True)
                            for kk in range(KK):
                                rhs = _view(c_b, kk * 224, [[1, 224]])
                                nc.tensor.matmul(
                                    od_b[:], id_sb[:], rhs,
                                    start=(kk == 0), stop=(kk == KK - 1))

                        # 1/D on ACT (slack engine): dr = exp(-ln(D)), SBUF
                        dr = dpool.tile([128, 224], F32, tag="dr")
                        if "norm" not in skip:
                            lnd = dpool.tile([128, 224], F32, tag="lnd")
                            nc.scalar.activation(
                                lnd[:], od_a[:, 224:448],
                                mybir.ActivationFunctionType.Ln)
                            nc.scalar.activation(
                                dr[:], lnd[:],
                                mybir.ActivationFunctionType.Exp, scale=-1.0)
                        pending_div = (od_a, od_b, dr, rb)

                        if sb == n_sub // 2 + 1:
                            for g in range(2):
                                nc.sync.dma_start(
                                    out=o_d[t, g, :, 0:2 * (n_sub // 2), :],
                                    in_=o_sb[g][:, 0:2 * (n_sub // 2), :])

                    if pending_div is not None and "norm" not in skip:
                        emit_divides(*pending_div)
                        pending_div = None
                    for g in range(2):
                        nc.sync.dma_start(
                            out=o_d[t, g, :, 2 * (n_sub // 2):, :],
                            in_=o_sb[g][:, 2 * (n_sub // 2):, :])

            # NB: tc.For_i emits raw-ISA register/branch ops this container's
            # walrus rejects ("ISA wrong length") -> python-unroll reps.
            for _ in range(reps):
                body()

    return nc


def _split_waits(nc, max_waits=1):
    """walrus in this container rejects >1 sync-wait per instruction;
    move extra waits onto preceding NOPs."""
    for fn in nc.m.functions:
        for blk in fn.blocks:
            insts = blk.instructions
            new_insts = []
            for inst in insts:
                si = getattr(inst, "sync_info", None)
                if si is not None and si.on_wait and len(si.on_wait) > max_waits:
                    waits = list(si.on_wait)
                    extra, keep = waits[:-max_waits], waits[-max_waits:]
                    k = 0
                    while extra:
                        chunk, extra = extra[:max_waits], extra[max_waits:]
                        new_insts.append(mybir.InstNoOp(
                            name=f"{inst.name}-ws{k}",
                            engine=inst.engine,
                            sync_info=mybir.SyncInfo(on_wait=chunk,
                                                     on_update=[]),
                            bass_nofuse=True,
                        ))
                        k += 1
                    inst.sync_info = mybir.SyncInfo(
                        on_wait=keep, on_update=list(si.on_update))
                new_insts.append(inst)
            blk.instructions.clear()
            blk.instructions.extend(new_insts)


# ------------------------------------------------------------- host helpers
def make_consts():
    bs = np.zeros((128, 128), np.float16)
    for u in range(8):
        bs[u * 16:(u + 1) * 16, u * 16:(u + 1) * 16] = np.float16(SCALE)
    return bs, np.eye(128, dtype=np.float16)


def _group_chans(o, h):
    """Flat-channel indices (len 128) for group (octet o, half h)."""
    p = np.arange(128)
    return (8 * o + p // 16) * 32 + 16 * h + (p % 16)


def shard_inputs(q, k, v):
    """Full [4,192,112,112] f32 -> per-core input maps."""
    qf = q.reshape(G, H, W).astype(np.float16)
    kp = np.pad(k.astype(np.float16),
                ((0, 0), (0, 0), (PAD, PAD), (PAD, PAD))).reshape(G, HP, WP)
    vp = np.pad(v.astype(np.float16),
                ((0, 0), (0, 0), (PAD, PAD), (PAD, PAD))).reshape(G, HP, WP)
    bs, id128 = make_consts()
    in_maps = []
    for c in range(N_CORES):
        qs = np.empty((N_PAIRS_PER_CORE, 128, 2, CHUNK_ROWS, W), np.float16)
        ks = np.empty((N_PAIRS_PER_CORE, 128, 2, CHUNK_ROWS + 4, WP),
                      np.float16)
        vs = np.empty_like(ks)
        for t in range(N_PAIRS_PER_CORE):
            P = c * N_PAIRS_PER_CORE + t
            o, e = divmod(P, 8)
            r0 = CHUNK_ROWS * e
            for h in range(2):
                ch = _group_chans(o, h)
                qs[t, :, h] = qf[ch, r0:r0 + CHUNK_ROWS, :]
                ks[t, :, h] = kp[ch, r0:r0 + CHUNK_ROWS + 4, :]
                vs[t, :, h] = vp[ch, r0:r0 + CHUNK_ROWS + 4, :]
        in_maps.append({
            "q": np.ascontiguousarray(qs),
            "k": np.ascontiguousarray(ks),
            "v": np.ascontiguousarray(vs),
            "bs": bs,
            "id128": id128,
        })
    return in_maps


def assemble_output(results):
    """Per-core 'o' [3,2,128,14,112] f32 -> full [4,112,112,192]."""
    oc = np.empty((G, H, W), np.float32)
    for c in range(N_CORES):
        for t in range(N_PAIRS_PER_CORE):
            P = c * N_PAIRS_PER_CORE + t
            o, e = divmod(P, 8)
            r0 = CHUNK_ROWS * e
            for h in range(2):
                ch = _group_chans(o, h)
                oc[ch, r0:r0 + CHUNK_ROWS, :] = results[c]["o"][t, h]
    return np.ascontiguousarray(
        oc.reshape(B, C, H, W).transpose(0, 2, 3, 1))


_NC_CACHE = {}


def kernel(q, k, v):
    key = "main"
    if key not in _NC_CACHE:
        nc_new = build_nc()
        _split_waits(nc_new)
        _NC_CACHE[key] = nc_new
    nc = _NC_CACHE[key]
    in_maps = shard_inputs(np.asarray(q), np.asarray(k), np.asarray(v))
    res = run_bass_kernel_spmd(nc, in_maps, list(range(N_CORES)))
    return assemble_output(res.results)
